# revision 32
# baseline (speedup 1.0000x reference)
"""LoRA first-layer MLP kernel for 8 Trainium2 NeuronCores.

Computation:
    W_eff = W0 + 2.0 * (B @ A)            # [4096, 1024]
    h     = relu(x @ W_eff^T + b0)        # [16384, 4096]
    out   = (h @ W2^T + b2).squeeze(-1)   # [16384]

Sharding: data-parallel over batch; each of the 8 cores handles 2048 rows of
x and replicates the weights. No collectives needed.

Per-core device kernel (fp32 data, fp32r matmul mode):
  - W0^T streamed to SBUF in [mc2(8), dc(8), 128, 512] blocks; the LoRA
    rank-16 correction 2*(B@A)^T is pre-added into each block on-device
    (PE matmul with zero-padded K=128 operands -> PSUM, DVE add),
    emitted just-in-time ahead of the tiles that read the block.
  - Layer 1: h^T[m, b] tiles [128, 512] accumulated on PE over 8 d-chunks
    (lhsT = W_eff^T slice [128d, 128m], rhs = x^T slice [128d, 512b]).
  - relu+bias on ScalarE (bias b0 is per-partition in this layout).
  - Layer 2 (sum_m W2[m]*h[m,b]) off the PE: even m-tiles accumulate on
    VectorE (scalar_tensor_tensor), odd tiles on GpSimdE (mul+add with a
    free-broadcast W2); final partition-reduce via two ones-vector
    matmuls per batch chunk, deferred into the next chunk's stream.
"""

import sys

sys.path.insert(0, "/opt/trn_rl_repo")

import numpy as np

import concourse.bacc as bacc
import concourse.bass as bass
import concourse.mybir as mybir
import concourse.tile as tile
from concourse.bass_utils import run_bass_kernel_spmd

F32 = mybir.dt.float32
F32R = mybir.dt.float32r

N_CORES = 8
B_FULL, D, M, R = 16384, 1024, 4096, 16
SCALING = 2.0
BS = B_FULL // N_CORES  # 2048 rows per core
NB = BS // 512  # 4 batch chunks per core
ND = D // 128  # 8 d-chunks
NM = M // 128  # 32 m-chunks
NM2 = M // 512  # 8 m-blocks of 512

_CACHE = {}


def _build_nc():
    nc = bacc.Bacc(
        "TRN2",
        target_bir_lowering=False,
        debug=False,
        num_devices=N_CORES,
    )
    xt = nc.dram_tensor("xt", [NB, 128, ND * 512], F32R, kind="ExternalInput").ap()
    w0t = nc.dram_tensor("w0t", [NM2, ND, 128, 512], F32R, kind="ExternalInput").ap()
    a2p = nc.dram_tensor("a2p", [128, D], F32R, kind="ExternalInput").ap()
    btp = nc.dram_tensor("btp", [128, M], F32R, kind="ExternalInput").ap()
    b0c = nc.dram_tensor("b0c", [128, NM], F32, kind="ExternalInput").ap()
    w2c = nc.dram_tensor("w2c", [128, NM], F32, kind="ExternalInput").ap()
    b2s = nc.dram_tensor("b2s", [1, 1], F32, kind="ExternalInput").ap()
    onesd = nc.dram_tensor("ones", [128, 1], F32R, kind="ExternalInput").ap()
    out = nc.dram_tensor("out", [1, BS], F32, kind="ExternalOutput").ap()

    RELU = mybir.ActivationFunctionType.Relu
    MULT = mybir.AluOpType.mult
    ADD = mybir.AluOpType.add

    with tile.TileContext(nc) as tc:
        with (
            tc.tile_pool(name="wp", bufs=1) as wp,
            tc.tile_pool(name="xp", bufs=2) as xp,
            tc.tile_pool(name="hb", bufs=4) as hb,
            tc.tile_pool(name="hw2", bufs=2) as hw2,
            tc.tile_pool(name="ab", bufs=2) as ab,
            tc.tile_pool(name="cp", bufs=1) as cp,
            tc.tile_pool(name="psh", bufs=3, space="PSUM") as psh,
            tc.tile_pool(name="pso", bufs=2, space="PSUM") as pso,
            tc.tile_pool(name="psl", bufs=3, space="PSUM") as psl,
        ):
            A2 = cp.tile([128, D], F32R, tag="a2")
            nc.sync.dma_start(out=A2[:], in_=a2p)

            # First x chunk interleaved with the first W m-block on sync.
            xb0 = xp.tile([128, ND * 512], F32R, tag="xb", name="xb0")

            def xb0_dma(dc):
                nc.sync.dma_start(
                    out=xb0[:, dc * 512 : (dc + 1) * 512],
                    in_=xt[0][:, dc * 512 : (dc + 1) * 512],
                )

            BT = cp.tile([128, M], F32R, tag="bt")
            for i in range(8):
                eng = nc.gpsimd if i % 2 == 0 else nc.scalar
                eng.dma_start(
                    out=BT[:, i * 512 : (i + 1) * 512],
                    in_=btp[:, i * 512 : (i + 1) * 512],
                )

            # Resident W_eff^T, laid out [mc2, dc, 512] along the free dim.
            W = wp.tile([128, NM2 * ND * 512], F32R, tag="w")

            def w_dma(mc2s):
                for mc2 in mc2s:
                    for dc in range(ND):
                        blk = (mc2 * ND + dc) * 512
                        nc.sync.dma_start(
                            out=W[:, blk : blk + 512], in_=w0t[mc2, dc]
                        )

            # xb0 slice dc, then W block (0, dc): compute needs both pairwise.
            for dc in range(ND):
                xb0_dma(dc)
                blk = dc * 512
                nc.sync.dma_start(out=W[:, blk : blk + 512], in_=w0t[0, dc])
            B0 = cp.tile([128, NM], F32, tag="b0")
            nc.sync.dma_start(out=B0[:], in_=b0c)
            W2 = cp.tile([128, NM], F32, tag="w2")
            nc.sync.dma_start(out=W2[:], in_=w2c)
            B2 = cp.tile([1, 1], F32, tag="b2")
            nc.sync.dma_start(out=B2[:], in_=b2s)
            ONES = cp.tile([128, 1], F32R, tag="ones")
            nc.sync.dma_start(out=ONES[:], in_=onesd)
            w_dma(range(1, NM2))

            def lora_block(mc2, dc):
                """W[:, blk:blk+512] += 2*(B@A)^T block, just-in-time."""
                blk = (mc2 * ND + dc) * 512
                lp = psl.tile([128, 512], F32, tag="lp")
                nc.tensor.matmul(
                    lp[:],
                    A2[:, dc * 128 : (dc + 1) * 128],
                    BT[:, mc2 * 512 : (mc2 + 1) * 512],
                    start=True,
                    stop=True,
                )
                nc.vector.tensor_add(
                    W[:, blk : blk + 512], W[:, blk : blk + 512], lp[:]
                )

            pending_reduce = []

            def emit_reduce(bc, acc_e, acc_o):
                op = pso.tile([1, 512], F32, tag="op")
                nc.tensor.matmul(op[:], ONES[:], acc_e[:], start=True, stop=False)
                nc.tensor.matmul(op[:], ONES[:], acc_o[:], start=False, stop=True)
                os_t = ab.tile([1, 512], F32, tag="os")
                nc.vector.tensor_scalar_add(os_t[:], op[:], B2[:, 0:1])
                nc.sync.dma_start(
                    out=out[:, bc * 512 : (bc + 1) * 512], in_=os_t[:]
                )

            for bc in range(NB):
                if bc == 0:
                    xb = xb0
                    # LoRA group 0 up front (paced by W DMA anyway)
                    for dc in range(ND):
                        lora_block(0, dc)
                else:
                    xb = xp.tile([128, ND * 512], F32R, tag="xb")
                    for dc in range(ND):
                        nc.sync.dma_start(
                            out=xb[:, dc * 512 : (dc + 1) * 512],
                            in_=xt[bc][:, dc * 512 : (dc + 1) * 512],
                        )
                acc_e = ab.tile([128, 512], F32R, tag="acce")
                acc_o = ab.tile([128, 512], F32R, tag="acco")
                for mc in range(NM):
                    mc2, j0 = mc // 4, (mc % 4) * 128
                    if bc == 0:
                        # prefetch next group's LoRA blocks, 2 per tile
                        g_next = mc // 4 + 1
                        if g_next < NM2:
                            for dc in (2 * (mc % 4), 2 * (mc % 4) + 1):
                                lora_block(g_next, dc)
                    if mc == 2 and pending_reduce:
                        emit_reduce(*pending_reduce.pop())
                    hp = psh.tile([128, 512], F32, tag="hp")
                    for dc in range(ND):
                        blk = (mc2 * ND + dc) * 512 + j0
                        nc.tensor.matmul(
                            hp[:],
                            W[:, blk : blk + 128],
                            xb[:, dc * 512 : (dc + 1) * 512],
                            start=(dc == 0),
                            stop=(dc == ND - 1),
                        )
                    h = hb.tile([128, 512], F32, tag="h")
                    nc.scalar.activation(h[:], hp[:], RELU, bias=B0[:, mc : mc + 1])
                    # acc += h * W2[m]: even tiles on VectorE (fused stt),
                    # odd tiles on GpSimd (mult into scratch, then add).
                    if mc % 2 == 0:
                        if mc == 0:
                            nc.vector.tensor_scalar_mul(
                                acc_e[:], h[:], W2[:, mc : mc + 1]
                            )
                        else:
                            nc.vector.scalar_tensor_tensor(
                                acc_e[:], h[:], W2[:, mc : mc + 1], acc_e[:],
                                MULT, ADD,
                            )
                    elif bc == NB - 1 and mc >= NM - 3:
                        nc.vector.scalar_tensor_tensor(
                            acc_o[:], h[:], W2[:, mc : mc + 1], acc_o[:],
                            MULT, ADD,
                        )
                    else:
                        w2b = W2[:, mc : mc + 1].broadcast_to([128, 512])
                        if mc == 1:
                            nc.gpsimd.tensor_mul(acc_o[:], h[:], w2b)
                        else:
                            hw = hw2.tile([128, 512], F32, tag="hw")
                            nc.gpsimd.tensor_mul(hw[:], h[:], w2b)
                            nc.gpsimd.tensor_add(acc_o[:], acc_o[:], hw[:])
                pending_reduce.append((bc, acc_e, acc_o))
            while pending_reduce:
                emit_reduce(*pending_reduce.pop(0))

    nc.compile()
    return nc


def _prep_in_maps(x, W0, b0, A, B, W2, b2):
    w0t_full = np.ascontiguousarray(W0.T).reshape(ND, 128, M)
    # -> [mc2, dc, 128, 512]
    w0t = np.ascontiguousarray(
        w0t_full.reshape(ND, 128, NM2, 512).transpose(2, 0, 1, 3)
    )
    a2p = np.zeros((128, D), dtype=np.float32)
    a2p[:R] = SCALING * A
    btp = np.zeros((128, M), dtype=np.float32)
    btp[:R] = B.T
    b0c = np.ascontiguousarray(b0.reshape(NM, 128).T)
    w2c = np.ascontiguousarray(W2[0].reshape(NM, 128).T)
    b2s = b2.reshape(1, 1).astype(np.float32)
    ones = np.ones((128, 1), dtype=np.float32)

    in_maps = []
    for c in range(N_CORES):
        xs = x[c * BS : (c + 1) * BS]  # [2048, 1024]
        # xt[bc, p, dc*512 + b] = xs[bc*512 + b, dc*128 + p]
        xt = np.ascontiguousarray(
            xs.reshape(NB, 512, ND, 128).transpose(0, 3, 2, 1).reshape(NB, 128, ND * 512)
        )
        in_maps.append(
            {
                "xt": xt,
                "w0t": w0t,
                "a2p": a2p,
                "btp": btp,
                "b0c": b0c,
                "w2c": w2c,
                "b2s": b2s,
                "ones": ones,
            }
        )
    return in_maps


def kernel(x, W0, b0, A, B, W2, b2, _trace=False, _trace_kwargs=None):
    x = np.asarray(x, dtype=np.float32)
    W0 = np.asarray(W0, dtype=np.float32)
    b0 = np.asarray(b0, dtype=np.float32)
    A = np.asarray(A, dtype=np.float32)
    B = np.asarray(B, dtype=np.float32)
    W2 = np.asarray(W2, dtype=np.float32)
    b2 = np.asarray(b2, dtype=np.float32)

    if "nc" not in _CACHE:
        _CACHE["nc"] = _build_nc()
    nc = _CACHE["nc"]

    in_maps = _prep_in_maps(x, W0, b0, A, B, W2, b2)
    res = run_bass_kernel_spmd(
        nc,
        in_maps,
        list(range(N_CORES)),
        trace=_trace,
        **(_trace_kwargs or {}),
    )
    out = np.concatenate([r["out"].reshape(BS) for r in res.results])
    if _trace:
        _CACHE["last_results"] = res
    return out.astype(np.float32)
